# revision 39
# baseline (speedup 1.0000x reference)
"""Trainium2 Bass kernel: multi-head attention (B=4, N=1024, D=1024, H=16)
distributed over 8 NeuronCores.

Sharding: (batch, head-group) -> one core each. Core (b, g) computes heads
g*8..g*8+7 of batch b for ALL 1024 queries: QKV projection restricted to its
8 heads' rows of w_qkv, full attention for those heads, and the partial
output projection against its 512 rows of w_out. The two partials per batch
are summed (plus b_out) on the host -- the standard row-parallel w_out
reduction. This removes the duplicated K/V projection of a (batch,
query-half) split: 528 matmuls/core instead of 667.
"""

import numpy as np
import concourse.bacc as bacc
import concourse.mybir as mybir
import concourse.tile as tile

dt = mybir.dt
F32, F16, BF16 = dt.float32, dt.float16, dt.bfloat16

B, N, D = 4, 1024, 1024
H = 16                 # total heads
HC = 8                 # heads per core
DH = 64                # head dim
F = HC * DH            # qkv features per section per core = 512
P = 128
DC = D // P            # 8 contraction chunks over d
NT = N // P            # 8 key-token tiles
FT = F // P            # 4 feature tiles per q/k section
SCALE = DH ** -0.5
AF = mybir.ActivationFunctionType


def _build_nc():
    nc = bacc.Bacc("TRN2", target_bir_lowering=False, debug=False)
    xT = nc.dram_tensor("xT", [D, N], F16, kind="ExternalInput")
    wq_t = nc.dram_tensor("wq_t", [FT, P, DC, P], F16, kind="ExternalInput")
    wk_t = nc.dram_tensor("wk_t", [FT, P, DC, P], F16, kind="ExternalInput")
    wv_t = nc.dram_tensor("wv_t", [P, DC, F], F16, kind="ExternalInput")
    wout_t = nc.dram_tensor("wout_t", [P, F // P, D], F16,
                            kind="ExternalInput")
    y = nc.dram_tensor("y", [N, D], F16, kind="ExternalOutput")

    with tile.TileContext(nc) as tc:
        with (
            tc.tile_pool(name="const", bufs=1) as cp,
            tc.tile_pool(name="work", bufs=2) as wp,
            tc.tile_pool(name="ps", bufs=1, space="PSUM") as pp,
        ):
            xT_sb = cp.tile([P, DC, N], F16)
            wq_sb = cp.tile([P, FT, DC, P], F16)
            wk_sb = cp.tile([P, FT, DC, P], F16)
            wv_sb = cp.tile([P, DC, F], F16)
            wout_sb = cp.tile([P, F // P, D], F16)
            qT_sb = cp.tile([P, FT, N], F16)
            kT_sb = cp.tile([P, FT, N], F16)
            v_sb = cp.tile([P, NT, HC, DH + 1], BF16)
            aT_sb = cp.tile([P, FT, N], F16)
            ones64 = cp.tile([1, DH], F16)
            nc.vector.memset(ones64, 1.0)
            nc.vector.memset(v_sb[:, :, :, DH:DH + 1], 1.0)

            # DMA order = need order. Host pre-tiles every weight tensor
            # so each transfer moves contiguous 2KB+ per-partition lines;
            # x goes chunk-by-chunk so the first q-projection group starts
            # as soon as wq block 0 + x chunk 0 land.
            nc.scalar.dma_start(wq_sb[:, 0, 0:1, :], wq_t.ap()[0][:, 0:1, :])
            nc.sync.dma_start(xT_sb[:, 0, 0:512], xT.ap()[0:P, 0:512])
            nc.scalar.dma_start(wq_sb[:, 0, 1:DC, :], wq_t.ap()[0][:, 1:DC, :])
            nc.scalar.dma_start(wk_sb[:, 0, :, :], wk_t.ap()[0])
            nc.sync.dma_start(xT_sb[:, 0, 512:1024], xT.ap()[0:P, 512:1024])
            for c in range(1, DC):
                nc.sync.dma_start(xT_sb[:, c, :], xT.ap()[c * P:(c + 1) * P, :])
            nc.scalar.dma_start(wv_sb[:, :, :], wv_t.ap()[:, :, :])
            for ft in range(1, FT):
                nc.scalar.dma_start(wq_sb[:, ft, :, :], wq_t.ap()[ft])
                nc.scalar.dma_start(wk_sb[:, ft, :, :], wk_t.ap()[ft])
            nc.scalar.dma_start(wout_sb[:, :, :], wout_t.ap()[:, :, :])

            # ---- projection groups --------------------------------------
            def q_proj(ft, jh):
                q_ps = pp.tile([P, N // 2], F32, tag="proj", bufs=2,
                               name=f"qps{ft}_{jh}")
                for c in range(DC):
                    nc.tensor.matmul(
                        q_ps[:, :],
                        lhsT=wq_sb[:, ft, c, :],
                        rhs=xT_sb[:, c, jh * 512:(jh + 1) * 512],
                        start=(c == 0), stop=(c == DC - 1),
                    )
                nc.vector.tensor_copy(
                    qT_sb[:, ft, jh * 512:(jh + 1) * 512], q_ps[:, :])

            def k_proj(ft, jh):
                k_ps = pp.tile([P, N // 2], F32, tag="proj", bufs=2,
                               name=f"kps{ft}_{jh}")
                for c in range(DC):
                    nc.tensor.matmul(
                        k_ps[:, :],
                        lhsT=wk_sb[:, ft, c, :],
                        rhs=xT_sb[:, c, jh * 512:(jh + 1) * 512],
                        start=(c == 0), stop=(c == DC - 1),
                    )
                nc.vector.tensor_copy(
                    kT_sb[:, ft, jh * 512:(jh + 1) * 512], k_ps[:, :])

            # Filler queue: projection / output-projection matmuls threaded
            # between attention matmuls so the in-order PE queue stays busy
            # while the ACT exp pipeline paces the attention stream. Each
            # step emits one matmul; the last step of a group also emits the
            # PSUM->SBUF eviction.
            def proj_steps(kind, a, b):
                state = {}
                def step(c):
                    if c == 0:
                        state["ps"] = pp.tile([P, 512], F32, tag="proj",
                                              bufs=2, name=f"{kind}ps{a}_{b}")
                    ps = state["ps"]
                    if kind == "q":
                        nc.tensor.matmul(
                            ps[:, :], lhsT=wq_sb[:, a, c, :],
                            rhs=xT_sb[:, c, b * 512:(b + 1) * 512],
                            start=(c == 0), stop=(c == DC - 1))
                        if c == DC - 1:
                            nc.vector.tensor_copy(
                                qT_sb[:, a, b * 512:(b + 1) * 512], ps[:, :])
                    elif kind == "k":
                        nc.tensor.matmul(
                            ps[:, :], lhsT=wk_sb[:, a, c, :],
                            rhs=xT_sb[:, c, b * 512:(b + 1) * 512],
                            start=(c == 0), stop=(c == DC - 1))
                        if c == DC - 1:
                            nc.vector.tensor_copy(
                                kT_sb[:, a, b * 512:(b + 1) * 512], ps[:, :])
                    else:  # v
                        nc.tensor.matmul(
                            ps[:, :], lhsT=xT_sb[:, c, a * P:(a + 1) * P],
                            rhs=wv_sb[:, c, :],
                            start=(c == 0), stop=(c == DC - 1))
                        if c == DC - 1:
                            nc.vector.tensor_copy(
                                v_sb[:, a, :, 0:DH],
                                ps[:, :].rearrange("p (h d) -> p h d", h=HC))
                return [lambda c=c: step(c) for c in range(DC)]

            # Output projection group (t4, j): y[t4 tokens, j features] =
            # sum_c aT[:, c, t4].T @ wout[:, c, j]. Emitted atomically (a
            # group that spans filler pops would deadlock the 2-buffer PSUM
            # rotation against the bc matmuls interleaved between pops).
            def out_group(t4, j, evict=None, ps_tag="proj"):
                if ps_tag == "s":
                    y_ps = pp.tile([P, 2, 512], F32, tag="s", bufs=2,
                                   name=f"yps{t4}_{j}")[:, 0, :]
                else:
                    y_ps = pp.tile([P, 512], F32, tag="proj", bufs=2,
                                   name=f"yps{t4}_{j}")
                for c in range(FT):
                    nc.tensor.matmul(
                        y_ps[:, :],
                        lhsT=aT_sb[:, c, t4 * P:(t4 + 1) * P],
                        rhs=wout_sb[:, c, j * 512:(j + 1) * 512],
                        start=(c == 0), stop=(c == FT - 1),
                    )
                ysb = wp.tile([P, 512], F16, tag="y", bufs=4,
                              name=f"ysb{t4}_{j}")
                if evict == "split":
                    for hh in range(2):
                        cols = slice(hh * 256, (hh + 1) * 256)
                        nc.vector.tensor_copy(ysb[:, cols], y_ps[:, cols])
                        nc.sync.dma_start(
                            y.ap()[t4 * P:(t4 + 1) * P,
                                   j * 512 + hh * 256:j * 512 + (hh + 1) * 256],
                            ysb[:, cols])
                else:
                    nc.vector.tensor_copy(ysb[:, :], y_ps[:, :])
                    nc.sync.dma_start(
                        y.ap()[t4 * P:(t4 + 1) * P, j * 512:(j + 1) * 512],
                        ysb[:, :])

            # Same group, split: chunks 0-2 now, chunk 3 + eviction later
            # (lets the tail overlap chunk-0-2 matmuls with norm(15)'s
            # reciprocal chain).
            def out_group_partial(t4, j):
                y_ps = pp.tile([P, 512], F32, tag="proj", bufs=2,
                               name=f"yps{t4}_{j}")
                for c in range(FT - 1):
                    nc.tensor.matmul(
                        y_ps[:, :],
                        lhsT=aT_sb[:, c, t4 * P:(t4 + 1) * P],
                        rhs=wout_sb[:, c, j * 512:(j + 1) * 512],
                        start=(c == 0), stop=False,
                    )
                def finish():
                    nc.tensor.matmul(
                        y_ps[:, :],
                        lhsT=aT_sb[:, FT - 1, t4 * P:(t4 + 1) * P],
                        rhs=wout_sb[:, FT - 1, j * 512:(j + 1) * 512],
                        start=False, stop=True,
                    )
                    ysb = wp.tile([P, 512], F16, tag="y", bufs=4,
                                  name=f"ysbp{t4}_{j}")
                    nc.vector.tensor_copy(ysb[:, :], y_ps[:, :])
                    nc.sync.dma_start(
                        y.ap()[t4 * P:(t4 + 1) * P, j * 512:(j + 1) * 512],
                        ysb[:, :])
                return finish

            # ---- filler schedule ----------------------------------------
            # (deadline, earliest, step): all steps with deadline <= u flush
            # before unit u's first scores matmul; pops never emit a step
            # before its `earliest` position (qh0 out-proj groups must wait
            # for norm(7)'s emission at unit 9 p1 = position 9.5).
            filler_units = []
            for nt in range(4):
                filler_units.append((0.5, 0, proj_steps("v", nt, 0)))
            for nt in range(4, NT):
                filler_units.append((1, 0, proj_steps("v", nt, 0)))
            filler_units.append((2, 0, proj_steps("k", 1, 0)))
            filler_units.append((2, 0, proj_steps("k", 1, 1)))
            filler_units.append((2, 0, proj_steps("q", 1, 0)))
            filler_units.append((4, 0, proj_steps("k", 2, 0)))
            filler_units.append((4, 0, proj_steps("k", 2, 1)))
            filler_units.append((4, 0, proj_steps("q", 2, 0)))
            filler_units.append((6, 0, proj_steps("k", 3, 0)))
            filler_units.append((6, 0, proj_steps("k", 3, 1)))
            filler_units.append((6, 0, proj_steps("q", 3, 0)))
            filler_units.append((9, 0, proj_steps("q", 1, 1)))
            filler_units.append((10, 0, proj_steps("q", 2, 1)))
            filler_units.append((11, 9.75, [lambda: out_group(0, 0)]))
            filler_units.append((11, 9.75, [lambda: out_group(0, 1)]))
            filler_units.append((12, 9.75, [lambda: out_group(1, 0)]))
            filler_units.append((12, 9.75, [lambda: out_group(1, 1)]))
            filler_units.append((13, 0, proj_steps("q", 3, 1)))
            filler_units.append((13, 9.75, [lambda: out_group(2, 0)]))
            filler_units.append((13.9, 13.3, [lambda: out_group(2, 1)]))
            # reserved for late-unit bubbles while exp/PV(15) drain
            filler_units.append((14.9, 14.3, [lambda: out_group(3, 0)]))
            filler_units.append((15.9, 15.5, [lambda: out_group(3, 1)]))
            filler_steps = [(dl, ea, s) for dl, ea, steps in filler_units
                            for s in steps]
            fill_pos = 0

            def flush_fillers(u):
                nonlocal fill_pos
                while (fill_pos < len(filler_steps)
                       and filler_steps[fill_pos][0] <= u):
                    filler_steps[fill_pos][2]()
                    fill_pos += 1

            def pop_filler(n, pos):
                nonlocal fill_pos
                k = 0
                while (k < n and fill_pos < len(filler_steps)
                       and filler_steps[fill_pos][1] <= pos):
                    filler_steps[fill_pos][2]()
                    fill_pos += 1
                    k += 1

            # ---- attention units ----------------------------------------
            # unit u = (qh, h): qh = u // 8, h = u % 8. Per unit: 8 scores
            # matmuls (4 two-bank PSUM pairs, each exp'd by one ACT over
            # 1024 columns), 8 PV matmuls of the PREVIOUS unit, and the
            # normalization finish of unit u-2. The softmax denominator
            # rides in v's 65th column (PV row 64).
            pend_pv = []       # (u, fn) deferred PV emission
            pend_norm = []     # deferred normalization finish

            def emit_pv(u):
                qh, h = u // 8, u % 8
                ft, r = h // 2, (h % 2) * DH
                pT = pT_tiles[u % 2]
                pv_ps = pp.tile([P, 512], F32, tag="pv", bufs=2, name=f"pv{u}")
                for c in range(NT):
                    nc.tensor.matmul(
                        pv_ps[0:DH + 1, :],
                        lhsT=v_sb[:, c, h, :],
                        rhs=pT[:, c, :],
                        start=(c == 0), stop=(c == NT - 1),
                    )
                srec32 = wp.tile([1, 512], F32, tag="sr32", bufs=2,
                                 name=f"sr32_{u}")
                srec16 = wp.tile([1, 512], F16, tag="sr16", bufs=2,
                                 name=f"sr16_{u}")
                den_sb = wp.tile([1, 512], F32, tag="den", bufs=2,
                                 name=f"den{u}")
                nc.vector.tensor_copy(den_sb[:, :], pv_ps[DH:DH + 1, :])
                nc.vector.reciprocal_approx_fast(
                    out=srec32[:, :], in_=den_sb[:, :])
                nc.vector.tensor_copy(srec16[:, :], srec32[:, :])

                def norm_finish(bc_tag="proj", u=u, ft=ft, r=r, qh=qh,
                                pv_ps=pv_ps, srec16=srec16):
                    bc_ps = pp.tile([P, 512], F32, tag=bc_tag, bufs=2,
                                    name=f"bc{u}")
                    nc.tensor.matmul(bc_ps[0:DH, :], lhsT=ones64[:, :],
                                     rhs=srec16[:, :], start=True, stop=True)
                    bc_sb = wp.tile([DH, 512], F32, tag="bc", bufs=2,
                                    name=f"bcs{u}")
                    nc.vector.tensor_copy(bc_sb[:, :], bc_ps[0:DH, :])
                    nc.vector.tensor_mul(
                        aT_sb[r:r + DH, ft, qh * 512:(qh + 1) * 512],
                        pv_ps[0:DH, :], bc_sb[:, :])
                pend_norm.append(norm_finish)

            pT_tiles = {}

            def unit(u):
                qh, h = u // 8, u % 8
                ft, r = h // 2, (h % 2) * DH
                flush_fillers(u)
                pT = wp.tile([P, NT, 512], BF16, tag="pT", bufs=2,
                             name=f"pT{u}")
                pT_tiles[u % 2] = pT
                for p in range(4):
                    s_ps = pp.tile([P, 2, 512], F32, tag="s", bufs=2,
                                   name=f"s{u}_{p}")
                    for i in range(2):
                        c = 2 * p + i
                        nc.tensor.matmul(
                            s_ps[:, i, :],
                            lhsT=kT_sb[r:r + DH, ft, c * P:(c + 1) * P],
                            rhs=qT_sb[r:r + DH, ft, qh * 512:(qh + 1) * 512],
                            start=True, stop=True,
                        )
                    nc.scalar.activation(pT[:, 2 * p:2 * p + 2, :],
                                         s_ps[:, :, :], AF.Exp, scale=SCALE)
                    if p == 0 and pend_pv:
                        pend_pv.pop(0)()
                    elif p == 1 and pend_norm and u >= 2:
                        pend_norm.pop(0)()
                    else:
                        pop_filler(2, u + (p + 1) / 4)
                pop_filler(2, u + 1)
                pend_pv.append(lambda u=u: emit_pv(u))

            # ---- preroll: unit 0 needs q(ft0, qh0) and k(ft0) ----------
            # All four ft0 projection groups interleave per x-chunk so each
            # chunk arrival feeds four matmuls (~full PE duty during the
            # DMA-paced ramp). Two accumulators borrow the scores pool,
            # which is idle until unit 0.
            q0_ps = pp.tile([P, 512], F32, tag="proj", bufs=2, name="qps0_0")
            k0_ps = pp.tile([P, 512], F32, tag="proj", bufs=2, name="kps0_0")
            k1_ps = pp.tile([P, 2, 512], F32, tag="s", bufs=2,
                            name="kps0_1")[:, 0, :]
            q1_ps = pp.tile([P, 2, 512], F32, tag="s", bufs=2,
                            name="qps0_1")[:, 0, :]
            for c in range(DC):
                nc.tensor.matmul(q0_ps[:, :], lhsT=wq_sb[:, 0, c, :],
                                 rhs=xT_sb[:, c, 0:512],
                                 start=(c == 0), stop=(c == DC - 1))
                nc.tensor.matmul(k0_ps[:, :], lhsT=wk_sb[:, 0, c, :],
                                 rhs=xT_sb[:, c, 0:512],
                                 start=(c == 0), stop=(c == DC - 1))
                nc.tensor.matmul(k1_ps[:, :], lhsT=wk_sb[:, 0, c, :],
                                 rhs=xT_sb[:, c, 512:1024],
                                 start=(c == 0), stop=(c == DC - 1))
                nc.tensor.matmul(q1_ps[:, :], lhsT=wq_sb[:, 0, c, :],
                                 rhs=xT_sb[:, c, 512:1024],
                                 start=(c == 0), stop=(c == DC - 1))
            nc.vector.tensor_copy(qT_sb[:, 0, 0:512], q0_ps[:, :])
            nc.vector.tensor_copy(kT_sb[:, 0, 0:512], k0_ps[:, :])
            nc.vector.tensor_copy(kT_sb[:, 0, 512:1024], k1_ps[:, :])
            nc.vector.tensor_copy(qT_sb[:, 0, 512:1024], q1_ps[:, :])

            for u in range(15):
                unit(u)

            # ---- unit 15 (hybrid): PV(15) inlined after its exp pairs so
            # the serial tail chain scores->exp->PV->norm shortens by ~2us.
            def unit15():
                qh, h = 1, 7
                ft, r = 3, 64
                flush_fillers(15)
                pT = wp.tile([P, NT, 512], BF16, tag="pT", bufs=2,
                             name="pT15")
                pT_tiles[1] = pT
                def pair(p):
                    s_ps = pp.tile([P, 2, 512], F32, tag="s", bufs=2,
                                   name=f"s15_{p}")
                    for i in range(2):
                        c = 2 * p + i
                        nc.tensor.matmul(
                            s_ps[:, i, :],
                            lhsT=kT_sb[r:r + DH, ft, c * P:(c + 1) * P],
                            rhs=qT_sb[r:r + DH, ft, qh * 512:(qh + 1) * 512],
                            start=True, stop=True,
                        )
                    nc.scalar.activation(pT[:, 2 * p:2 * p + 2, :],
                                         s_ps[:, :, :], AF.Exp, scale=SCALE)
                pair(0)
                pend_pv.pop(0)()      # PV(14) + its reciprocal
                pair(1)
                pend_norm.pop(0)()    # norm finish (13); frees pv(13)
                pair(2)
                pv_ps = pp.tile([P, 512], F32, tag="pv", bufs=2, name="pv15")
                for c in range(2):
                    nc.tensor.matmul(pv_ps[0:DH + 1, :],
                                     lhsT=v_sb[:, c, h, :], rhs=pT[:, c, :],
                                     start=(c == 0), stop=False)
                pair(3)
                for c in range(2, 4):
                    nc.tensor.matmul(pv_ps[0:DH + 1, :],
                                     lhsT=v_sb[:, c, h, :], rhs=pT[:, c, :],
                                     start=False, stop=False)
                pop_filler(1, 16)     # reserved out groups pad the exp drain
                for c in range(4, 6):
                    nc.tensor.matmul(pv_ps[0:DH + 1, :],
                                     lhsT=v_sb[:, c, h, :], rhs=pT[:, c, :],
                                     start=False, stop=False)
                pop_filler(1, 16)
                for c in range(6, 8):
                    nc.tensor.matmul(pv_ps[0:DH + 1, :],
                                     lhsT=v_sb[:, c, h, :], rhs=pT[:, c, :],
                                     start=False, stop=(c == 7))
                pop_filler(1, 16)
                den_sb = wp.tile([1, 512], F32, tag="den", bufs=2,
                                 name="den15")
                nc.vector.tensor_copy(den_sb[:, :], pv_ps[DH:DH + 1, :])
                srec32 = wp.tile([1, 512], F32, tag="sr32", bufs=2,
                                 name="sr32_15")
                srec16 = wp.tile([1, 512], F16, tag="sr16", bufs=2,
                                 name="sr16_15")
                nc.vector.reciprocal_approx_fast(
                    out=srec32[:, :], in_=den_sb[:, :])
                nc.vector.tensor_copy(srec16[:, :], srec32[:, :])
                def norm15(bc_tag="proj"):
                    bc_ps = pp.tile([P, 512], F32, tag=bc_tag, bufs=2,
                                    name="bc15")
                    nc.tensor.matmul(bc_ps[0:DH, :], lhsT=ones64[:, :],
                                     rhs=srec16[:, :], start=True, stop=True)
                    bc_sb = wp.tile([DH, 512], F32, tag="bc", bufs=2,
                                    name="bcs15")
                    nc.vector.tensor_copy(bc_sb[:, :], bc_ps[0:DH, :])
                    nc.vector.tensor_mul(
                        aT_sb[r:r + DH, ft, qh * 512:(qh + 1) * 512],
                        pv_ps[0:DH, :], bc_sb[:, :])
                pend_norm.append(norm15)
            unit15()

            # ---- tail ---------------------------------------------------
            flush_fillers(100)
            pend_norm.pop(0)()        # norm finish (14)
            # Two qh1 out groups open with chunks 0-2 (heads 0-5, norms done
            # by unit 15) while norm(15)'s reciprocal drains; its bc matmul
            # borrows the pv PSUM tag so the two open groups can't deadlock
            # the proj pool rotation.
            fin0 = out_group_partial(4, 0)
            fin1 = out_group_partial(4, 1)
            pend_norm.pop(0)(bc_tag="pv")   # norm finish (15)
            fin0()
            fin1()
            for t4 in range(5, 8):
                for j in range(2):
                    out_group(t4, j, evict="split" if t4 == 7 else None,
                              ps_tag="s" if j == 1 else "proj")
    nc.compile()
    return nc


def _make_in_maps(x, w_qkv, w_out, b_out):
    f16 = np.float16
    in_maps = []
    for core in range(8):
        b, g = core // 2, core % 2
        rs = slice(g * F, (g + 1) * F)
        wq = w_qkv[rs].astype(f16)                       # [F, D]
        wk = w_qkv[D + g * F:D + (g + 1) * F].astype(f16)
        wv = w_qkv[2 * D + g * F:2 * D + (g + 1) * F].astype(f16)
        # [FT, P, DC, P]: t[ft, p, c, f] = w[ft*P + f, c*P + p]
        def tile4(w):
            return np.ascontiguousarray(
                w.reshape(FT, P, DC, P).transpose(0, 3, 2, 1))
        in_maps.append({
            "xT": np.ascontiguousarray(x[b].astype(f16).T),
            "wq_t": tile4(wq),
            "wk_t": tile4(wk),
            # [P, DC, F]: t[p, c, f] = wv[f, c*P + p]
            "wv_t": np.ascontiguousarray(
                wv.reshape(F, DC, P).transpose(2, 1, 0)),
            # [P, F//P, D]: t[p, c, ff] = w_out[ff, g*F + c*P + p]
            "wout_t": np.ascontiguousarray(
                w_out[:, rs].astype(f16).T.reshape(FT, P, D)
                .transpose(1, 0, 2)),
        })
    return in_maps


def _assemble(results, b_out):
    y = np.empty((B, N, D), dtype=np.float32)
    for b in range(B):
        y[b] = (results[2 * b]["y"].astype(np.float32)
                + results[2 * b + 1]["y"].astype(np.float32))
    y += b_out.astype(np.float32)
    return y


_NC_CACHE = {}


def kernel(x, w_qkv, w_out, b_out):
    import numpy as _np
    from concourse.bass_utils import run_bass_kernel_spmd
    if "nc" not in _NC_CACHE:
        _NC_CACHE["nc"] = _build_nc()
    nc = _NC_CACHE["nc"]
    in_maps = _make_in_maps(_np.asarray(x), _np.asarray(w_qkv),
                            _np.asarray(w_out), _np.asarray(b_out))
    res = run_bass_kernel_spmd(nc, in_maps, list(range(8)))
    return _assemble(res.results, _np.asarray(b_out))


# revision 40
# speedup vs baseline: 1.0013x; 1.0013x over previous
"""Trainium2 Bass kernel: multi-head attention (B=4, N=1024, D=1024, H=16)
distributed over 8 NeuronCores.

Sharding: (batch, head-group) -> one core each. Core (b, g) computes heads
g*8..g*8+7 of batch b for ALL 1024 queries: QKV projection restricted to its
8 heads' rows of w_qkv, full attention for those heads, and the partial
output projection against its 512 rows of w_out. The two partials per batch
are summed (plus b_out) on the host -- the standard row-parallel w_out
reduction. This removes the duplicated K/V projection of a (batch,
query-half) split: 528 matmuls/core instead of 667.
"""

import numpy as np
import concourse.bacc as bacc
import concourse.mybir as mybir
import concourse.tile as tile

dt = mybir.dt
F32, F16, BF16 = dt.float32, dt.float16, dt.bfloat16

B, N, D = 4, 1024, 1024
H = 16                 # total heads
HC = 8                 # heads per core
DH = 64                # head dim
F = HC * DH            # qkv features per section per core = 512
P = 128
DC = D // P            # 8 contraction chunks over d
NT = N // P            # 8 key-token tiles
FT = F // P            # 4 feature tiles per q/k section
SCALE = DH ** -0.5
AF = mybir.ActivationFunctionType


def _build_nc():
    nc = bacc.Bacc("TRN2", target_bir_lowering=False, debug=False)
    xT = nc.dram_tensor("xT", [D, N], F16, kind="ExternalInput")
    wq_t = nc.dram_tensor("wq_t", [FT, P, DC, P], F16, kind="ExternalInput")
    wk_t = nc.dram_tensor("wk_t", [FT, P, DC, P], F16, kind="ExternalInput")
    wv_t = nc.dram_tensor("wv_t", [P, DC, F], F16, kind="ExternalInput")
    wout_t = nc.dram_tensor("wout_t", [P, F // P, D], F16,
                            kind="ExternalInput")
    y = nc.dram_tensor("y", [N, D], F16, kind="ExternalOutput")

    with tile.TileContext(nc) as tc:
        with (
            tc.tile_pool(name="const", bufs=1) as cp,
            tc.tile_pool(name="work", bufs=2) as wp,
            tc.tile_pool(name="ps", bufs=1, space="PSUM") as pp,
        ):
            xT_sb = cp.tile([P, DC, N], F16)
            wq_sb = cp.tile([P, FT, DC, P], F16)
            wk_sb = cp.tile([P, FT, DC, P], F16)
            wv_sb = cp.tile([P, DC, F], F16)
            wout_sb = cp.tile([P, F // P, D], F16)
            qT_sb = cp.tile([P, FT, N], F16)
            kT_sb = cp.tile([P, FT, N], F16)
            v_sb = cp.tile([P, NT, HC, DH + 1], BF16)
            aT_sb = cp.tile([P, FT, N], F16)
            ones64 = cp.tile([1, DH], F16)
            nc.vector.memset(ones64, 1.0)
            nc.vector.memset(v_sb[:, :, :, DH:DH + 1], 1.0)

            # DMA order = need order. Host pre-tiles every weight tensor
            # so each transfer moves contiguous 2KB+ per-partition lines;
            # x goes chunk-by-chunk so the first q-projection group starts
            # as soon as wq block 0 + x chunk 0 land.
            nc.scalar.dma_start(wq_sb[:, 0, 0:1, :], wq_t.ap()[0][:, 0:1, :])
            nc.sync.dma_start(xT_sb[:, 0, 0:512], xT.ap()[0:P, 0:512])
            nc.scalar.dma_start(wq_sb[:, 0, 1:DC, :], wq_t.ap()[0][:, 1:DC, :])
            nc.scalar.dma_start(wk_sb[:, 0, :, :], wk_t.ap()[0])
            nc.sync.dma_start(xT_sb[:, 0, 512:1024], xT.ap()[0:P, 512:1024])
            for c in range(1, DC):
                nc.sync.dma_start(xT_sb[:, c, :], xT.ap()[c * P:(c + 1) * P, :])
            nc.scalar.dma_start(wv_sb[:, :, :], wv_t.ap()[:, :, :])
            for ft in range(1, FT):
                nc.scalar.dma_start(wq_sb[:, ft, :, :], wq_t.ap()[ft])
                nc.scalar.dma_start(wk_sb[:, ft, :, :], wk_t.ap()[ft])
            nc.scalar.dma_start(wout_sb[:, :, :], wout_t.ap()[:, :, :])

            # ---- projection groups --------------------------------------
            def q_proj(ft, jh):
                q_ps = pp.tile([P, N // 2], F32, tag="proj", bufs=2,
                               name=f"qps{ft}_{jh}")
                for c in range(DC):
                    nc.tensor.matmul(
                        q_ps[:, :],
                        lhsT=wq_sb[:, ft, c, :],
                        rhs=xT_sb[:, c, jh * 512:(jh + 1) * 512],
                        start=(c == 0), stop=(c == DC - 1),
                    )
                nc.vector.tensor_copy(
                    qT_sb[:, ft, jh * 512:(jh + 1) * 512], q_ps[:, :])

            def k_proj(ft, jh):
                k_ps = pp.tile([P, N // 2], F32, tag="proj", bufs=2,
                               name=f"kps{ft}_{jh}")
                for c in range(DC):
                    nc.tensor.matmul(
                        k_ps[:, :],
                        lhsT=wk_sb[:, ft, c, :],
                        rhs=xT_sb[:, c, jh * 512:(jh + 1) * 512],
                        start=(c == 0), stop=(c == DC - 1),
                    )
                nc.vector.tensor_copy(
                    kT_sb[:, ft, jh * 512:(jh + 1) * 512], k_ps[:, :])

            # Filler queue: projection / output-projection matmuls threaded
            # between attention matmuls so the in-order PE queue stays busy
            # while the ACT exp pipeline paces the attention stream. Each
            # step emits one matmul; the last step of a group also emits the
            # PSUM->SBUF eviction.
            def proj_steps(kind, a, b):
                state = {}
                def step(c):
                    if c == 0:
                        state["ps"] = pp.tile([P, 512], F32, tag="proj",
                                              bufs=2, name=f"{kind}ps{a}_{b}")
                    ps = state["ps"]
                    if kind == "q":
                        nc.tensor.matmul(
                            ps[:, :], lhsT=wq_sb[:, a, c, :],
                            rhs=xT_sb[:, c, b * 512:(b + 1) * 512],
                            start=(c == 0), stop=(c == DC - 1))
                        if c == DC - 1:
                            nc.vector.tensor_copy(
                                qT_sb[:, a, b * 512:(b + 1) * 512], ps[:, :])
                    elif kind == "k":
                        nc.tensor.matmul(
                            ps[:, :], lhsT=wk_sb[:, a, c, :],
                            rhs=xT_sb[:, c, b * 512:(b + 1) * 512],
                            start=(c == 0), stop=(c == DC - 1))
                        if c == DC - 1:
                            nc.vector.tensor_copy(
                                kT_sb[:, a, b * 512:(b + 1) * 512], ps[:, :])
                    else:  # v
                        nc.tensor.matmul(
                            ps[:, :], lhsT=xT_sb[:, c, a * P:(a + 1) * P],
                            rhs=wv_sb[:, c, :],
                            start=(c == 0), stop=(c == DC - 1))
                        if c == DC - 1:
                            nc.vector.tensor_copy(
                                v_sb[:, a, :, 0:DH],
                                ps[:, :].rearrange("p (h d) -> p h d", h=HC))
                return [lambda c=c: step(c) for c in range(DC)]

            # Output projection group (t4, j): y[t4 tokens, j features] =
            # sum_c aT[:, c, t4].T @ wout[:, c, j]. Emitted atomically (a
            # group that spans filler pops would deadlock the 2-buffer PSUM
            # rotation against the bc matmuls interleaved between pops).
            def out_group(t4, j, evict=None, ps_tag="proj"):
                if ps_tag == "s":
                    y_ps = pp.tile([P, 2, 512], F32, tag="s", bufs=2,
                                   name=f"yps{t4}_{j}")[:, 0, :]
                else:
                    y_ps = pp.tile([P, 512], F32, tag="proj", bufs=2,
                                   name=f"yps{t4}_{j}")
                for c in range(FT):
                    nc.tensor.matmul(
                        y_ps[:, :],
                        lhsT=aT_sb[:, c, t4 * P:(t4 + 1) * P],
                        rhs=wout_sb[:, c, j * 512:(j + 1) * 512],
                        start=(c == 0), stop=(c == FT - 1),
                    )
                ysb = wp.tile([P, 512], F16, tag="y", bufs=4,
                              name=f"ysb{t4}_{j}")
                if evict == "split":
                    for hh in range(2):
                        cols = slice(hh * 256, (hh + 1) * 256)
                        nc.vector.tensor_copy(ysb[:, cols], y_ps[:, cols])
                        nc.sync.dma_start(
                            y.ap()[t4 * P:(t4 + 1) * P,
                                   j * 512 + hh * 256:j * 512 + (hh + 1) * 256],
                            ysb[:, cols])
                else:
                    nc.vector.tensor_copy(ysb[:, :], y_ps[:, :])
                    nc.sync.dma_start(
                        y.ap()[t4 * P:(t4 + 1) * P, j * 512:(j + 1) * 512],
                        ysb[:, :])

            # Same group, split: chunks 0-2 now, chunk 3 + eviction later
            # (lets the tail overlap chunk-0-2 matmuls with norm(15)'s
            # reciprocal chain).
            def out_group_partial(t4, j):
                y_ps = pp.tile([P, 512], F32, tag="proj", bufs=2,
                               name=f"yps{t4}_{j}")
                for c in range(FT - 1):
                    nc.tensor.matmul(
                        y_ps[:, :],
                        lhsT=aT_sb[:, c, t4 * P:(t4 + 1) * P],
                        rhs=wout_sb[:, c, j * 512:(j + 1) * 512],
                        start=(c == 0), stop=False,
                    )
                def finish():
                    nc.tensor.matmul(
                        y_ps[:, :],
                        lhsT=aT_sb[:, FT - 1, t4 * P:(t4 + 1) * P],
                        rhs=wout_sb[:, FT - 1, j * 512:(j + 1) * 512],
                        start=False, stop=True,
                    )
                    ysb = wp.tile([P, 512], F16, tag="y", bufs=4,
                                  name=f"ysbp{t4}_{j}")
                    nc.vector.tensor_copy(ysb[:, :], y_ps[:, :])
                    nc.sync.dma_start(
                        y.ap()[t4 * P:(t4 + 1) * P, j * 512:(j + 1) * 512],
                        ysb[:, :])
                return finish

            # ---- filler schedule ----------------------------------------
            # (deadline, earliest, step): all steps with deadline <= u flush
            # before unit u's first scores matmul; pops never emit a step
            # before its `earliest` position (qh0 out-proj groups must wait
            # for norm(7)'s emission at unit 9 p1 = position 9.5).
            filler_units = []
            for nt in range(4):
                filler_units.append((0.5, 0, proj_steps("v", nt, 0)))
            for nt in range(4, NT):
                filler_units.append((1, 0, proj_steps("v", nt, 0)))
            filler_units.append((2, 0, proj_steps("k", 1, 0)))
            filler_units.append((2, 0, proj_steps("k", 1, 1)))
            filler_units.append((2, 0, proj_steps("q", 1, 0)))
            filler_units.append((4, 0, proj_steps("k", 2, 0)))
            filler_units.append((4, 0, proj_steps("k", 2, 1)))
            filler_units.append((4, 0, proj_steps("q", 2, 0)))
            filler_units.append((6, 0, proj_steps("k", 3, 0)))
            filler_units.append((6, 0, proj_steps("k", 3, 1)))
            filler_units.append((6, 0, proj_steps("q", 3, 0)))
            filler_units.append((9, 0, proj_steps("q", 1, 1)))
            filler_units.append((10, 0, proj_steps("q", 2, 1)))
            filler_units.append((11, 9.75, [lambda: out_group(0, 0)]))
            filler_units.append((11, 9.75, [lambda: out_group(0, 1)]))
            filler_units.append((12, 9.75, [lambda: out_group(1, 0)]))
            filler_units.append((12, 9.75, [lambda: out_group(1, 1)]))
            filler_units.append((13, 0, proj_steps("q", 3, 1)))
            filler_units.append((13, 9.75, [lambda: out_group(2, 0)]))
            filler_units.append((13.9, 13.3, [lambda: out_group(2, 1)]))
            # reserved for late-unit bubbles while exp/PV(15) drain
            filler_units.append((14.9, 14.3, [lambda: out_group(3, 0)]))
            filler_units.append((15.9, 15.5, [lambda: out_group(3, 1)]))
            filler_steps = [(dl, ea, s) for dl, ea, steps in filler_units
                            for s in steps]
            fill_pos = 0

            def flush_fillers(u):
                nonlocal fill_pos
                while (fill_pos < len(filler_steps)
                       and filler_steps[fill_pos][0] <= u):
                    filler_steps[fill_pos][2]()
                    fill_pos += 1

            def pop_filler(n, pos):
                nonlocal fill_pos
                k = 0
                while (k < n and fill_pos < len(filler_steps)
                       and filler_steps[fill_pos][1] <= pos):
                    filler_steps[fill_pos][2]()
                    fill_pos += 1
                    k += 1

            # ---- attention units ----------------------------------------
            # unit u = (qh, h): qh = u // 8, h = u % 8. Per unit: 8 scores
            # matmuls (4 two-bank PSUM pairs, each exp'd by one ACT over
            # 1024 columns), 8 PV matmuls of the PREVIOUS unit, and the
            # normalization finish of unit u-2. The softmax denominator
            # rides in v's 65th column (PV row 64).
            pend_pv = []       # (u, fn) deferred PV emission
            pend_norm = []     # deferred normalization finish

            def emit_pv(u):
                qh, h = u // 8, u % 8
                ft, r = h // 2, (h % 2) * DH
                pT = pT_tiles[u % 2]
                pv_ps = pp.tile([P, 512], F32, tag="pv", bufs=2, name=f"pv{u}")
                for c in range(NT):
                    nc.tensor.matmul(
                        pv_ps[0:DH + 1, :],
                        lhsT=v_sb[:, c, h, :],
                        rhs=pT[:, c, :],
                        start=(c == 0), stop=(c == NT - 1),
                    )
                srec32 = wp.tile([1, 512], F32, tag="sr32", bufs=2,
                                 name=f"sr32_{u}")
                srec16 = wp.tile([1, 512], F16, tag="sr16", bufs=2,
                                 name=f"sr16_{u}")
                den_sb = wp.tile([1, 512], F32, tag="den", bufs=2,
                                 name=f"den{u}")
                nc.vector.tensor_copy(den_sb[:, :], pv_ps[DH:DH + 1, :])
                nc.vector.reciprocal_approx_fast(
                    out=srec32[:, :], in_=den_sb[:, :])
                nc.vector.tensor_copy(srec16[:, :], srec32[:, :])

                def norm_finish(bc_tag="proj", u=u, ft=ft, r=r, qh=qh,
                                pv_ps=pv_ps, srec16=srec16):
                    bc_ps = pp.tile([P, 512], F32, tag=bc_tag, bufs=2,
                                    name=f"bc{u}")
                    nc.tensor.matmul(bc_ps[0:DH, :], lhsT=ones64[:, :],
                                     rhs=srec16[:, :], start=True, stop=True)
                    bc_sb = wp.tile([DH, 512], F32, tag="bc", bufs=2,
                                    name=f"bcs{u}")
                    nc.vector.tensor_copy(bc_sb[:, :], bc_ps[0:DH, :])
                    nc.vector.tensor_mul(
                        aT_sb[r:r + DH, ft, qh * 512:(qh + 1) * 512],
                        pv_ps[0:DH, :], bc_sb[:, :])
                pend_norm.append(norm_finish)

            pT_tiles = {}

            def unit(u):
                qh, h = u // 8, u % 8
                ft, r = h // 2, (h % 2) * DH
                flush_fillers(u)
                pT = wp.tile([P, NT, 512], BF16, tag="pT", bufs=2,
                             name=f"pT{u}")
                pT_tiles[u % 2] = pT
                for p in range(4):
                    s_ps = pp.tile([P, 2, 512], F32, tag="s", bufs=2,
                                   name=f"s{u}_{p}")
                    for i in range(2):
                        c = 2 * p + i
                        nc.tensor.matmul(
                            s_ps[:, i, :],
                            lhsT=kT_sb[r:r + DH, ft, c * P:(c + 1) * P],
                            rhs=qT_sb[r:r + DH, ft, qh * 512:(qh + 1) * 512],
                            start=True, stop=True,
                        )
                    nc.scalar.activation(pT[:, 2 * p:2 * p + 2, :],
                                         s_ps[:, :, :], AF.Exp, scale=SCALE)
                    if p == 0 and pend_pv:
                        pend_pv.pop(0)()
                    elif p == 1 and pend_norm and u >= 2:
                        pend_norm.pop(0)()
                    else:
                        pop_filler(2, u + (p + 1) / 4)
                pop_filler(2, u + 1)
                pend_pv.append(lambda u=u: emit_pv(u))

            # ---- preroll: unit 0 needs q(ft0, qh0) and k(ft0) ----------
            # All four ft0 projection groups interleave per x-chunk so each
            # chunk arrival feeds four matmuls (~full PE duty during the
            # DMA-paced ramp). Two accumulators borrow the scores pool,
            # which is idle until unit 0.
            q0_ps = pp.tile([P, 512], F32, tag="proj", bufs=2, name="qps0_0")
            k0_ps = pp.tile([P, 512], F32, tag="proj", bufs=2, name="kps0_0")
            k1_ps = pp.tile([P, 2, 512], F32, tag="s", bufs=2,
                            name="kps0_1")[:, 0, :]
            q1_ps = pp.tile([P, 2, 512], F32, tag="s", bufs=2,
                            name="qps0_1")[:, 0, :]
            for c in range(DC):
                nc.tensor.matmul(q0_ps[:, :], lhsT=wq_sb[:, 0, c, :],
                                 rhs=xT_sb[:, c, 0:512],
                                 start=(c == 0), stop=(c == DC - 1))
                nc.tensor.matmul(k0_ps[:, :], lhsT=wk_sb[:, 0, c, :],
                                 rhs=xT_sb[:, c, 0:512],
                                 start=(c == 0), stop=(c == DC - 1))
                nc.tensor.matmul(k1_ps[:, :], lhsT=wk_sb[:, 0, c, :],
                                 rhs=xT_sb[:, c, 512:1024],
                                 start=(c == 0), stop=(c == DC - 1))
                nc.tensor.matmul(q1_ps[:, :], lhsT=wq_sb[:, 0, c, :],
                                 rhs=xT_sb[:, c, 512:1024],
                                 start=(c == 0), stop=(c == DC - 1))
            # Split evictions across DVE and the (still idle) ACT engine so
            # unit 0's scores aren't gated on a serial 4-deep DVE chain.
            nc.vector.tensor_copy(kT_sb[:, 0, 0:512], k0_ps[:, :])
            nc.scalar.copy(kT_sb[:, 0, 512:1024], k1_ps[:, :])
            nc.vector.tensor_copy(qT_sb[:, 0, 0:512], q0_ps[:, :])
            nc.scalar.copy(qT_sb[:, 0, 512:1024], q1_ps[:, :])

            for u in range(15):
                unit(u)

            # ---- unit 15 (hybrid): PV(15) inlined after its exp pairs so
            # the serial tail chain scores->exp->PV->norm shortens by ~2us.
            def unit15():
                qh, h = 1, 7
                ft, r = 3, 64
                flush_fillers(15)
                pT = wp.tile([P, NT, 512], BF16, tag="pT", bufs=2,
                             name="pT15")
                pT_tiles[1] = pT
                def pair(p):
                    s_ps = pp.tile([P, 2, 512], F32, tag="s", bufs=2,
                                   name=f"s15_{p}")
                    for i in range(2):
                        c = 2 * p + i
                        nc.tensor.matmul(
                            s_ps[:, i, :],
                            lhsT=kT_sb[r:r + DH, ft, c * P:(c + 1) * P],
                            rhs=qT_sb[r:r + DH, ft, qh * 512:(qh + 1) * 512],
                            start=True, stop=True,
                        )
                    nc.scalar.activation(pT[:, 2 * p:2 * p + 2, :],
                                         s_ps[:, :, :], AF.Exp, scale=SCALE)
                pair(0)
                pend_pv.pop(0)()      # PV(14) + its reciprocal
                pair(1)
                pend_norm.pop(0)()    # norm finish (13); frees pv(13)
                pair(2)
                pv_ps = pp.tile([P, 512], F32, tag="pv", bufs=2, name="pv15")
                for c in range(2):
                    nc.tensor.matmul(pv_ps[0:DH + 1, :],
                                     lhsT=v_sb[:, c, h, :], rhs=pT[:, c, :],
                                     start=(c == 0), stop=False)
                pair(3)
                for c in range(2, 4):
                    nc.tensor.matmul(pv_ps[0:DH + 1, :],
                                     lhsT=v_sb[:, c, h, :], rhs=pT[:, c, :],
                                     start=False, stop=False)
                pop_filler(1, 16)     # reserved out groups pad the exp drain
                for c in range(4, 6):
                    nc.tensor.matmul(pv_ps[0:DH + 1, :],
                                     lhsT=v_sb[:, c, h, :], rhs=pT[:, c, :],
                                     start=False, stop=False)
                pop_filler(1, 16)
                for c in range(6, 8):
                    nc.tensor.matmul(pv_ps[0:DH + 1, :],
                                     lhsT=v_sb[:, c, h, :], rhs=pT[:, c, :],
                                     start=False, stop=(c == 7))
                pop_filler(1, 16)
                den_sb = wp.tile([1, 512], F32, tag="den", bufs=2,
                                 name="den15")
                nc.vector.tensor_copy(den_sb[:, :], pv_ps[DH:DH + 1, :])
                srec32 = wp.tile([1, 512], F32, tag="sr32", bufs=2,
                                 name="sr32_15")
                srec16 = wp.tile([1, 512], F16, tag="sr16", bufs=2,
                                 name="sr16_15")
                nc.vector.reciprocal_approx_fast(
                    out=srec32[:, :], in_=den_sb[:, :])
                nc.vector.tensor_copy(srec16[:, :], srec32[:, :])
                def norm15(bc_tag="proj"):
                    bc_ps = pp.tile([P, 512], F32, tag=bc_tag, bufs=2,
                                    name="bc15")
                    nc.tensor.matmul(bc_ps[0:DH, :], lhsT=ones64[:, :],
                                     rhs=srec16[:, :], start=True, stop=True)
                    bc_sb = wp.tile([DH, 512], F32, tag="bc", bufs=2,
                                    name="bcs15")
                    nc.vector.tensor_copy(bc_sb[:, :], bc_ps[0:DH, :])
                    nc.vector.tensor_mul(
                        aT_sb[r:r + DH, ft, qh * 512:(qh + 1) * 512],
                        pv_ps[0:DH, :], bc_sb[:, :])
                pend_norm.append(norm15)
            unit15()

            # ---- tail ---------------------------------------------------
            flush_fillers(100)
            pend_norm.pop(0)()        # norm finish (14)
            # Two qh1 out groups open with chunks 0-2 (heads 0-5, norms done
            # by unit 15) while norm(15)'s reciprocal drains; its bc matmul
            # borrows the pv PSUM tag so the two open groups can't deadlock
            # the proj pool rotation.
            fin0 = out_group_partial(4, 0)
            fin1 = out_group_partial(4, 1)
            pend_norm.pop(0)(bc_tag="pv")   # norm finish (15)
            fin0()
            fin1()
            for t4 in range(5, 8):
                for j in range(2):
                    out_group(t4, j, evict="split" if t4 == 7 else None,
                              ps_tag="s" if j == 1 else "proj")
    nc.compile()
    return nc


def _make_in_maps(x, w_qkv, w_out, b_out):
    f16 = np.float16
    in_maps = []
    for core in range(8):
        b, g = core // 2, core % 2
        rs = slice(g * F, (g + 1) * F)
        wq = w_qkv[rs].astype(f16)                       # [F, D]
        wk = w_qkv[D + g * F:D + (g + 1) * F].astype(f16)
        wv = w_qkv[2 * D + g * F:2 * D + (g + 1) * F].astype(f16)
        # [FT, P, DC, P]: t[ft, p, c, f] = w[ft*P + f, c*P + p]
        def tile4(w):
            return np.ascontiguousarray(
                w.reshape(FT, P, DC, P).transpose(0, 3, 2, 1))
        in_maps.append({
            "xT": np.ascontiguousarray(x[b].astype(f16).T),
            "wq_t": tile4(wq),
            "wk_t": tile4(wk),
            # [P, DC, F]: t[p, c, f] = wv[f, c*P + p]
            "wv_t": np.ascontiguousarray(
                wv.reshape(F, DC, P).transpose(2, 1, 0)),
            # [P, F//P, D]: t[p, c, ff] = w_out[ff, g*F + c*P + p]
            "wout_t": np.ascontiguousarray(
                w_out[:, rs].astype(f16).T.reshape(FT, P, D)
                .transpose(1, 0, 2)),
        })
    return in_maps


def _assemble(results, b_out):
    y = np.empty((B, N, D), dtype=np.float32)
    for b in range(B):
        y[b] = (results[2 * b]["y"].astype(np.float32)
                + results[2 * b + 1]["y"].astype(np.float32))
    y += b_out.astype(np.float32)
    return y


_NC_CACHE = {}


def kernel(x, w_qkv, w_out, b_out):
    import numpy as _np
    from concourse.bass_utils import run_bass_kernel_spmd
    if "nc" not in _NC_CACHE:
        _NC_CACHE["nc"] = _build_nc()
    nc = _NC_CACHE["nc"]
    in_maps = _make_in_maps(_np.asarray(x), _np.asarray(w_qkv),
                            _np.asarray(w_out), _np.asarray(b_out))
    res = run_bass_kernel_spmd(nc, in_maps, list(range(8)))
    return _assemble(res.results, _np.asarray(b_out))
